# revision 30
# baseline (speedup 1.0000x reference)
"""Trainium2 Bass kernel for a single-step attention GRU decoder.

Math (reference):
  embedded = emb[idx]                                  [1, H]
  attn_w   = softmax(cat(embedded, h) @ attn_W.T + attn_b)      [1, L]
  attn_app = attn_w @ encoder_output                   [1, H]
  x        = relu(cat(embedded, attn_app) @ comb_W.T + comb_b)  [1, H]
  gi = x @ W_ih.T + b_ih ; gh = h @ W_hh.T + b_hh      [1, 3H]
  r = sig(gi_r+gh_r); z = sig(gi_z+gh_z); n = tanh(gi_n + r*gh_n)
  h_new = (1-z)*n + z*h
  out = log_softmax(h_new @ out_W.T + out_b)           [1, V]

Sharding over 8 NeuronCores (tensor-parallel, memory-bound):
  - out_W / out_b sharded over vocab (6400 padded rows per core).
  - attention rows (L) sharded 64/core, encoder rows likewise.
  - comb_W replicated (8.4MB) so every core computes the full x vector
    locally: its embedded-half runs in the dead window before the first
    collective wakes up, removing one AllGather round from the chain.
  - GRU weights sharded by hidden slice inside each gate (128/core).
  - The sequential chain is stitched with 3 small AllGathers
    (attention partials, h_new, local sum-exp).
  - log-softmax computed locally per shard with a global logsumexp.
"""

import os
import numpy as np

H = 1024
V = 50257
L = 512
NCORES = 8
LC = L // NCORES          # 64 attention rows per core
HC = H // NCORES          # 128 hidden rows per core
VC = 6400                 # padded vocab rows per core (8*6400 = 51200 >= V)
VT = VC // 128            # 50 out_W tiles of [128, 1024] per core
W_BUFS = 16               # out_W streaming pool depth (13 MB SBUF)
NEG = -1.0e30             # bias padding -> exp() == 0

_CACHE = {}


def _build():
    from concourse import bass, bacc, mybir, tile
    from concourse.bass import _add_dep_helper

    fp32 = mybir.dt.float32
    nc = bacc.Bacc("TRN2", target_bir_lowering=False, debug=False,
                   num_devices=NCORES)

    # ---- kernel I/O (per-core shards supplied via in_maps) ----
    emb_row = nc.dram_tensor("emb_row", [1, H], fp32, kind="ExternalInput")
    h_full = nc.dram_tensor("h_full", [1, H], fp32, kind="ExternalInput")
    h_sl = nc.dram_tensor("h_sl", [HC, 1], fp32, kind="ExternalInput")
    enc_c = nc.dram_tensor("enc_c", [LC, H], fp32, kind="ExternalInput")
    attn_w_c = nc.dram_tensor("attn_w_c", [LC, 2 * H], fp32, kind="ExternalInput")
    attn_b_c = nc.dram_tensor("attn_b_c", [LC, 1], fp32, kind="ExternalInput")
    comb_we = nc.dram_tensor("comb_we", [H, H], fp32, kind="ExternalInput")
    comb_wa = nc.dram_tensor("comb_wa", [H, H], fp32, kind="ExternalInput")
    cbt = nc.dram_tensor("cbt", [128, 8], fp32, kind="ExternalInput")
    w_ih_c = nc.dram_tensor("w_ih_c", [3, HC, H], fp32, kind="ExternalInput")
    b_ih_c = nc.dram_tensor("b_ih_c", [3, HC], fp32, kind="ExternalInput")
    w_hh_c = nc.dram_tensor("w_hh_c", [3, HC, H], fp32, kind="ExternalInput")
    b_hh_c = nc.dram_tensor("b_hh_c", [3, HC], fp32, kind="ExternalInput")
    out_w_c = nc.dram_tensor("out_w_c", [VC, H], fp32, kind="ExternalInput")
    out_b_c = nc.dram_tensor("out_b_c", [128, VT], fp32, kind="ExternalInput")
    eye_in = nc.dram_tensor("eye_in", [128, 128], fp32, kind="ExternalInput")

    logits_out = nc.dram_tensor("logits_out", [128, VT], fp32, kind="ExternalOutput")
    hnew_out = nc.dram_tensor("hnew_out", [H], fp32, kind="ExternalOutput")
    attnw_out = nc.dram_tensor("attnw_out", [L], fp32, kind="ExternalOutput")

    AG1W = 1096               # [attn_partial(1024) | sumexp(1) | pad(7) | exp(64)]
    mult = mybir.AluOpType.mult
    add = mybir.AluOpType.add
    AF = mybir.ActivationFunctionType

    with tile.TileContext(nc) as tc:
        with (
            tc.tile_pool(name="cst", bufs=1) as cst,
            tc.tile_pool(name="wpool", bufs=W_BUFS) as wpool,
            tc.tile_pool(name="cwe_p", bufs=2) as cwe_p,
            tc.tile_pool(name="cwa_p", bufs=8) as cwa_p,
            tc.tile_pool(name="psum", bufs=1, space="PSUM") as psum,
            tc.tile_pool(name="dram", bufs=1, space="DRAM") as dram,
        ):
            # ---- collective bounce buffers (DRAM) ----
            ag1_in = dram.tile([AG1W], fp32, tag="ag1i")
            ag1_out = dram.tile([NCORES * AG1W], fp32, tag="ag1o",
                                addr_space="Shared")
            ag3_in = dram.tile([HC], fp32, tag="ag3i")
            ag3_out = dram.tile([H], fp32, tag="ag3o", addr_space="Shared")
            ag4_in = dram.tile([8], fp32, tag="ag4i")
            ag4_out = dram.tile([64], fp32, tag="ag4o", addr_space="Shared")

            # ---- persistent SBUF tiles ----
            cat3 = cst.tile([128, 3 * H], fp32, tag="cat3")   # [emb | h | attn] bcast
            scratch = cst.tile([128, 2 * H], fp32, tag="scratch")
            enc_sb = cst.tile([LC, H + 1], fp32, tag="enc")
            aw_sb = cst.tile([LC, 2 * H], fp32, tag="aw")
            ab_sb = cst.tile([LC, 1], fp32, tag="ab")
            cbt_sb = cst.tile([128, 8], fp32, tag="cbt")
            wih_sb = cst.tile([HC, 3, H], fp32, tag="wih")
            whh_sb = cst.tile([HC, 3, H], fp32, tag="whh")
            bih_sb = cst.tile([HC, 3], fp32, tag="bih")
            bhh_sb = cst.tile([HC, 3], fp32, tag="bhh")
            hsl_sb = cst.tile([HC, 1], fp32, tag="hsl")
            ob_sb = cst.tile([128, VT], fp32, tag="ob")
            row_sb = cst.tile([1, H], fp32, tag="row")        # emb/h staging row
            row2_sb = cst.tile([1, H], fp32, tag="row2")
            xrow = cst.tile([1, H], fp32, tag="xrow")
            hrow = cst.tile([1, H], fp32, tag="hrow")
            xb = cst.tile([128, H], fp32, tag="xb")           # x broadcast
            hb = cst.tile([128, H], fp32, tag="hb")           # h_new broadcast
            ones8 = cst.tile([8, 1], fp32, tag="ones8")
            ones128 = cst.tile([128, 1], fp32, tag="ones128")
            sc_acc = cst.tile([LC, 1], fp32, tag="scacc")
            exps = cst.tile([LC, 1], fp32, tag="exps")
            ar1sb = cst.tile([1, 1096], fp32, tag="ar1sb")
            se_sb = cst.tile([1, 8], fp32, tag="sesb")
            agsb = cst.tile([NCORES, AG1W], fp32, tag="agsb")
            rs_sb = cst.tile([1, 1], fp32, tag="rs")
            rs8_sb = cst.tile([8, 1], fp32, tag="rs8")
            attnn_sb = cst.tile([1, H], fp32, tag="attnn")
            awout_sb = cst.tile([NCORES, LC], fp32, tag="awout")
            comb1 = cst.tile([128, 8], fp32, tag="comb1")
            comb2 = cst.tile([128, 8], fp32, tag="comb2")
            x_sb = cst.tile([128, 8], fp32, tag="xsb")
            gh_sb = cst.tile([HC, 3], fp32, tag="gh")
            gi_sb = cst.tile([HC, 3], fp32, tag="gi")
            gh_acc = cst.tile([HC, 3], fp32, tag="ghacc")
            gi_acc = cst.tile([HC, 3], fp32, tag="giacc")
            lg_acc = cst.tile([128, VT], fp32, tag="lgacc")
            r_g = cst.tile([HC, 1], fp32, tag="rg")
            z_g = cst.tile([HC, 1], fp32, tag="zg")
            t_g = cst.tile([HC, 1], fp32, tag="tg")
            n_g = cst.tile([HC, 1], fp32, tag="ng")
            u_g = cst.tile([HC, 1], fp32, tag="ug")
            hn_sl = cst.tile([HC, 1], fp32, tag="hnsl")
            lg_sb = cst.tile([128, VT], fp32, tag="lg")
            elg_sb = cst.tile([128, VT], fp32, tag="elg")
            sexp_p = cst.tile([128, 1], fp32, tag="sexpp")
            s8_sb = cst.tile([1, 8], fp32, tag="s8")
            stot_sb = cst.tile([1, 1], fp32, tag="stot")
            lse_sb = cst.tile([1, 1], fp32, tag="lse")
            lse128 = cst.tile([128, 1], fp32, tag="lse128")
            outsb = cst.tile([128, VT], fp32, tag="outsb")
            warm_sb = cst.tile([1, 2], fp32, tag="warm")
            eye_sb = cst.tile([128, 128], fp32, tag="eye")
            hnst_sb = cst.tile([1, HC], fp32, tag="hnst")

            psum1 = psum.tile([1, 1536], fp32, tag="p1")
            psum2 = psum.tile([1, 1536], fp32, tag="p2")
            psum3 = psum.tile([1, 1], fp32, tag="p3")
            psum_t = psum.tile([1, 128], fp32, tag="pt")

            # ---- phase 0: load weights / stage broadcasts ----
            nc.scalar.dma_start(out=row_sb[:, :], in_=emb_row[:, :])
            nc.scalar.dma_start(out=row2_sb[:, :], in_=h_full[:, :])
            nc.gpsimd.partition_broadcast(cat3[:, 0:H], row_sb[0:1, :])
            nc.gpsimd.partition_broadcast(cat3[:, H:2 * H], row2_sb[0:1, :])
            nc.sync.dma_start(out=enc_sb[:, 0:H], in_=enc_c[:, :])
            nc.vector.memset(enc_sb[:, H:H + 1], 1.0)
            nc.sync.dma_start(out=aw_sb[:, :], in_=attn_w_c[:, :])
            nc.sync.dma_start(out=ab_sb[:, :], in_=attn_b_c[:, :])
            nc.sync.dma_start(out=cbt_sb[:, :], in_=cbt[:, :])
            wih_ld = nc.sync.dma_start(out=wih_sb[:, :, :],
                              in_=w_ih_c.ap().rearrange("g p c -> p g c"))
            whh_ld = nc.sync.dma_start(out=whh_sb[:, :, :],
                              in_=w_hh_c.ap().rearrange("g p c -> p g c"))
            nc.sync.dma_start(out=bih_sb[:, :],
                              in_=b_ih_c.ap().rearrange("g p -> p g"))
            nc.sync.dma_start(out=bhh_sb[:, :],
                              in_=b_hh_c.ap().rearrange("g p -> p g"))
            nc.scalar.dma_start(out=hsl_sb[:, :], in_=h_sl[:, :])
            ob_ld = nc.sync.dma_start(out=ob_sb[:, :], in_=out_b_c[:, :])
            nc.sync.dma_start(out=eye_sb[:, :], in_=eye_in[:, :])
            nc.vector.memset(ones8[:, :], 1.0)
            nc.vector.memset(ones128[:, :], 1.0)
            # prewarm activation tables off the critical path
            nc.vector.memset(warm_sb[:, 0:1], 1.0)
            for wf in (AF.Exp, AF.Relu, AF.Sigmoid, AF.Tanh, AF.Ln):
                nc.scalar.activation(warm_sb[:, 1:2], warm_sb[:, 0:1], wf)

            # ---- phase 1: attention scores -> exp -> partial attn ----
            nc.vector.scalar_tensor_tensor(
                out=scratch[0:LC, 0:2 * H], in0=aw_sb[:, :], scalar=1.0,
                in1=cat3[0:LC, 0:2 * H], op0=mult, op1=mult,
                accum_out=sc_acc[:, :])
            nc.scalar.activation(exps[:, :], sc_acc[:, :], AF.Exp,
                                 bias=ab_sb[:, :])
            # comb embedded-half (replicated) + gh: fill the dead window
            # before the first collective wakes up
            for b in range(8):
                cwe_t = cwe_p.tile([128, H], fp32, tag="cwe")
                ld = nc.sync.dma_start(out=cwe_t[:, :],
                                       in_=comb_we[b * 128:(b + 1) * 128, :])
                nc.vector.scalar_tensor_tensor(
                    out=scratch[:, 0:H], in0=cwe_t[:, :], scalar=1.0,
                    in1=cat3[:, 0:H], op0=mult, op1=mult,
                    accum_out=comb1[:, b:b + 1])
            for g in range(3):
                nc.vector.scalar_tensor_tensor(
                    out=scratch[:, 0:H], in0=whh_sb[:, g, :], scalar=1.0,
                    in1=cat3[:, H:2 * H], op0=mult, op1=mult,
                    accum_out=gh_acc[:, g:g + 1])
            nc.vector.scalar_tensor_tensor(
                out=gh_sb[:, :], in0=gh_acc[:, :], scalar=1.0,
                in1=bhh_sb[:, :], op0=mult, op1=add)
            nc.vector.scalar_tensor_tensor(
                out=comb1[:, :], in0=comb1[:, :], scalar=1.0,
                in1=cbt_sb[:, :], op0=mult, op1=add)

            # psum1[0, 0:1025] = exp_scores.T @ [enc | 1]
            for n0, n1 in ((0, 512), (512, 1024), (1024, 1025)):
                nc.tensor.matmul(psum1[0:1, n0:n1], exps[:, 0:1],
                                 enc_sb[:, n0:n1], start=True, stop=True)

            nc.tensor.transpose(psum_t[0:1, 0:LC], exps[:, :],
                                 eye_sb[0:LC, 0:LC])
            nc.scalar.copy(ar1sb[:, 0:1025], psum1[0:1, 0:1025])
            nc.vector.memset(ar1sb[:, 1025:1032], 0.0)
            nc.scalar.copy(ar1sb[:, 1032:1096], psum_t[0:1, 0:LC])
            gate = nc.scalar.dma_start(
                out=ag1_in[:].rearrange("(one f) -> one f", one=1),
                in_=ar1sb[:, :])
            nc.gpsimd.collective_compute(
                "AllGather", mybir.AluOpType.bypass,
                replica_groups=[list(range(NCORES))],
                ins=[ag1_in[:].opt()], outs=[ag1_out[:].opt()])
            nc.scalar.dma_start(out=agsb[:, :],
                                in_=ag1_out[:].rearrange("(r f) -> r f", r=NCORES))

            # rank-sum of partials: psum2[0, :1025] = ones.T @ agsb[:, :1025]
            for n0, n1 in ((0, 512), (512, 1024), (1024, 1025)):
                nc.tensor.matmul(psum2[0:1, n0:n1], ones8[:, :],
                                 agsb[:, n0:n1], start=True, stop=True)

            nc.vector.reciprocal(rs_sb[:, :], psum2[0:1, 1024:1025])
            # normalized attn_applied -> broadcast into cat3[:, 2H:3H]
            nc.vector.tensor_scalar_mul(attnn_sb[:, :],
                                        psum2[0:1, 0:H], rs_sb[:, :])
            nc.gpsimd.partition_broadcast(cat3[:, 2 * H:3 * H], attnn_sb[0:1, :])

            # ---- phase 2: combine -> relu -> x slice -> AllGather ----
            # attn-half of comb (replicated weights): full x computed locally
            for b in range(8):
                cwa_t = cwa_p.tile([128, H], fp32, tag="cwa")
                nc.scalar.dma_start(out=cwa_t[:, :],
                                    in_=comb_wa[b * 128:(b + 1) * 128, :])
                nc.vector.scalar_tensor_tensor(
                    out=scratch[:, 0:H], in0=cwa_t[:, :], scalar=1.0,
                    in1=cat3[:, 2 * H:3 * H], op0=mult, op1=mult,
                    accum_out=comb2[:, b:b + 1])
            nc.vector.scalar_tensor_tensor(
                out=comb2[:, :], in0=comb2[:, :], scalar=1.0,
                in1=comb1[:, :], op0=mult, op1=add)
            nc.scalar.activation(x_sb[:, :], comb2[:, :], AF.Relu)
            # x [128, 8] partition-major -> xrow [1, 1024] via PE transposes
            for b in range(8):
                pslot = psum_t[0:1, 0:128] if b % 2 == 0 else psum2[0:1, 0:128]
                nc.tensor.transpose(pslot, x_sb[:, b:b + 1], eye_sb[:, :])
                dst = xrow[0:1, b * 128:(b + 1) * 128]
                if b % 2 == 0:
                    nc.scalar.copy(dst, pslot)
                else:
                    nc.vector.tensor_copy(dst, pslot)
            nc.gpsimd.partition_broadcast(xb[:, :], xrow[0:1, :])

            # ---- phase 3: gi, gates, h_new slice -> AllGather ----
            for g in range(3):
                nc.vector.scalar_tensor_tensor(
                    out=scratch[:, 0:H], in0=wih_sb[:, g, :], scalar=1.0,
                    in1=xb[:, :], op0=mult, op1=mult,
                    accum_out=gi_acc[:, g:g + 1])
            nc.vector.scalar_tensor_tensor(
                out=gi_sb[:, :], in0=gi_acc[:, :], scalar=1.0,
                in1=bih_sb[:, :], op0=mult, op1=add)
            nc.scalar.activation(r_g[:, :], gi_sb[:, 0:1], AF.Sigmoid,
                                 bias=gh_sb[:, 0:1])
            nc.scalar.activation(z_g[:, :], gi_sb[:, 1:2], AF.Sigmoid,
                                 bias=gh_sb[:, 1:2])
            nc.vector.tensor_scalar_mul(t_g[:, :], r_g[:, :], gh_sb[:, 2:3])
            nc.scalar.activation(n_g[:, :], gi_sb[:, 2:3], AF.Tanh,
                                 bias=t_g[:, :])
            nc.vector.tensor_scalar_sub(u_g[:, :], hsl_sb[:, :], n_g[:, :])
            # h_new = z*(h-n) + n
            nc.vector.scalar_tensor_tensor(
                out=hn_sl[:, :], in0=z_g[:, :], scalar=u_g[:, :],
                in1=n_g[:, :], op0=mult, op1=add)
            nc.tensor.transpose(psum_t[0:1, 0:HC], hn_sl[:, :], eye_sb[:, :])
            nc.scalar.copy(hnst_sb[:, :], psum_t[0:1, 0:HC])
            nc.scalar.dma_start(
                out=ag3_in[:].rearrange("(one f) -> one f", one=1),
                in_=hnst_sb[:, :])
            nc.gpsimd.collective_compute(
                "AllGather", mybir.AluOpType.bypass,
                replica_groups=[list(range(NCORES))],
                ins=[ag3_in[:].opt()], outs=[ag3_out[:].opt()])
            nc.scalar.dma_start(out=hrow[0:1, :],
                                in_=ag3_out[:].rearrange("(one f) -> one f", one=1))
            nc.gpsimd.partition_broadcast(hb[:, :], hrow[0:1, :])

            # ---- phase 4: vocab-shard logits via fused mul+reduce ----
            for t in range(VT):
                wt = wpool.tile([128, H], fp32, tag="wt")
                wld = nc.sync.dma_start(out=wt[:, :],
                                        in_=out_w_c[t * 128:(t + 1) * 128, :])
                _add_dep_helper(wld.ins, gate.ins, True,
                                "stream after AG1 input to keep queues clear")
                nc.vector.scalar_tensor_tensor(
                    out=scratch[:, 0:H], in0=wt[:, :], scalar=1.0,
                    in1=hb[:, :], op0=mult, op1=mult,
                    accum_out=lg_acc[:, t:t + 1])
            nc.vector.scalar_tensor_tensor(
                out=lg_sb[:, :], in0=lg_acc[:, :], scalar=1.0,
                in1=ob_sb[:, :], op0=mult, op1=add)

            for ld in (wih_ld, whh_ld, ob_ld):
                _add_dep_helper(ld.ins, gate.ins, True,
                                "bulk load after AG1 input to keep queues clear")

            # attn_weights / h_new outputs (created late => low priority,
            # they fill idle engine time during the logits phase)
            nc.gpsimd.partition_broadcast(rs8_sb[:, :], rs_sb[0:1, :])
            nc.vector.tensor_scalar_mul(awout_sb[:, :],
                                        agsb[:, 1032:1096], rs8_sb[:, :])
            nc.scalar.dma_start(out=attnw_out.ap().rearrange("(p f) -> p f", p=NCORES),
                                in_=awout_sb[:, :])
            nc.scalar.dma_start(out=hnew_out[:], in_=ag3_out[:])

            # local sum(exp(logits)) -> AllGather -> global logsumexp
            nc.scalar.activation(elg_sb[:, :], lg_sb[:, :], AF.Exp,
                                 accum_out=sexp_p[:, :])
            nc.tensor.matmul(psum3[0:1, 0:1], ones128[:, :], sexp_p[:, :],
                             start=True, stop=True)
            nc.scalar.copy(se_sb[:, 0:1], psum3[0:1, 0:1])
            nc.vector.memset(se_sb[:, 1:8], 0.0)
            nc.scalar.dma_start(
                out=ag4_in[:].rearrange("(one f) -> one f", one=1),
                in_=se_sb[:, :])
            nc.gpsimd.collective_compute(
                "AllGather", mybir.AluOpType.bypass,
                replica_groups=[list(range(NCORES))],
                ins=[ag4_in[:].opt()], outs=[ag4_out[:].opt()])
            # rank r's local sum-exp sits at element r*8 of ag4_out
            nc.scalar.dma_start(
                out=s8_sb[0:1, :],
                in_=ag4_out[:].rearrange("(r f) -> f r", f=8)[0:1, :])
            nc.vector.reduce_sum(stot_sb[:, :], s8_sb[:, :],
                                 axis=mybir.AxisListType.X)
            nc.scalar.activation(lse_sb[:, :], stot_sb[:, :], AF.Ln)
            nc.gpsimd.partition_broadcast(lse128[:, :], lse_sb[0:1, :])
            nc.vector.tensor_scalar_sub(outsb[:, :], lg_sb[:, :], lse128[:, :])
            nc.scalar.dma_start(out=logits_out[:, :], in_=outsb[:, :])

    nc.compile()
    return nc


def _get_nc():
    if "nc" not in _CACHE:
        _CACHE["nc"] = _build()
    return _CACHE["nc"]


def _shard_inputs(input_idx, hidden, encoder_output, emb, attn_W, attn_b,
                  comb_W, comb_b, W_ih, W_hh, b_ih, b_hh, out_W, out_b):
    f32 = np.float32
    idx = int(np.asarray(input_idx).reshape(-1)[0])
    emb_row = np.ascontiguousarray(np.asarray(emb)[idx]).reshape(1, H).astype(f32)
    h = np.asarray(hidden, dtype=f32).reshape(1, H)
    enc = np.asarray(encoder_output, dtype=f32)
    attn_W = np.asarray(attn_W, dtype=f32)
    attn_b = np.asarray(attn_b, dtype=f32)
    comb_W = np.asarray(comb_W, dtype=f32)
    comb_b = np.asarray(comb_b, dtype=f32)
    W_ih = np.asarray(W_ih, dtype=f32).reshape(3, H, H)
    W_hh = np.asarray(W_hh, dtype=f32).reshape(3, H, H)
    b_ih = np.asarray(b_ih, dtype=f32).reshape(3, H)
    b_hh = np.asarray(b_hh, dtype=f32).reshape(3, H)
    out_W = np.asarray(out_W, dtype=f32)
    out_b = np.asarray(out_b, dtype=f32)

    comb_we = np.ascontiguousarray(comb_W[:, :H])
    comb_wa = np.ascontiguousarray(comb_W[:, H:])
    cbt_host = np.ascontiguousarray(comb_b.reshape(8, 128).T)

    out_W_pad = np.zeros((NCORES * VC, H), dtype=f32)
    out_W_pad[:V] = out_W
    out_b_pad = np.full((NCORES * VC,), NEG, dtype=f32)
    out_b_pad[:V] = out_b

    in_maps = []
    for c in range(NCORES):
        ls, le = c * LC, (c + 1) * LC
        hs, he = c * HC, (c + 1) * HC
        vs, ve = c * VC, (c + 1) * VC
        in_maps.append({
            "emb_row": emb_row,
            "h_full": h,
            "h_sl": np.ascontiguousarray(h[0, hs:he]).reshape(HC, 1),
            "enc_c": np.ascontiguousarray(enc[ls:le]),
            "attn_w_c": np.ascontiguousarray(attn_W[ls:le]),
            "attn_b_c": np.ascontiguousarray(attn_b[ls:le]).reshape(LC, 1),
            "comb_we": comb_we,
            "comb_wa": comb_wa,
            "cbt": cbt_host,
            "w_ih_c": np.ascontiguousarray(W_ih[:, hs:he, :]),
            "b_ih_c": np.ascontiguousarray(b_ih[:, hs:he]),
            "w_hh_c": np.ascontiguousarray(W_hh[:, hs:he, :]),
            "b_hh_c": np.ascontiguousarray(b_hh[:, hs:he]),
            "out_w_c": np.ascontiguousarray(out_W_pad[vs:ve]),
            "out_b_c": np.ascontiguousarray(out_b_pad[vs:ve].reshape(VT, 128).T),
            "eye_in": np.eye(128, dtype=f32),
        })
    return in_maps


def kernel(**inputs):
    from concourse import bass_utils

    nc = _get_nc()
    in_maps = _shard_inputs(**inputs)
    trace = bool(int(os.environ.get("KERNEL_PROFILE", "0")))
    res = bass_utils.run_bass_kernel_spmd(
        nc, in_maps, core_ids=list(range(NCORES)), trace=trace)
    if trace:
        _CACHE["last_result"] = res

    logits = np.concatenate(
        [res.results[c]["logits_out"].reshape(128, VT).T.reshape(-1)
         for c in range(NCORES)])
    out = logits[:V].reshape(1, V).astype(np.float32)
    h_new = res.results[0]["hnew_out"].reshape(1, 1, H).astype(np.float32)
    attn_w = res.results[0]["attnw_out"].reshape(1, L).astype(np.float32)
    return out, h_new, attn_w


# revision 31
# speedup vs baseline: 1.2554x; 1.2554x over previous
"""Trainium2 Bass kernel for a single-step attention GRU decoder.

Math (reference):
  embedded = emb[idx]                                  [1, H]
  attn_w   = softmax(cat(embedded, h) @ attn_W.T + attn_b)      [1, L]
  attn_app = attn_w @ encoder_output                   [1, H]
  x        = relu(cat(embedded, attn_app) @ comb_W.T + comb_b)  [1, H]
  gi = x @ W_ih.T + b_ih ; gh = h @ W_hh.T + b_hh      [1, 3H]
  r = sig(gi_r+gh_r); z = sig(gi_z+gh_z); n = tanh(gi_n + r*gh_n)
  h_new = (1-z)*n + z*h
  out = log_softmax(h_new @ out_W.T + out_b)           [1, V]

Sharding over 8 NeuronCores (tensor-parallel, memory-bound):
  - out_W / out_b sharded over vocab (6400 padded rows per core).
  - attention rows (L) sharded 64/core, encoder rows likewise.
  - comb_W replicated (8.4MB) so every core computes the full x vector
    locally: its embedded-half runs in the dead window before the first
    collective wakes up, removing one AllGather round from the chain.
  - GRU weights sharded by hidden slice inside each gate (128/core).
  - The sequential chain is stitched with 3 small AllGathers
    (attention partials, h_new, local sum-exp).
  - log-softmax computed locally per shard with a global logsumexp.
"""

import os
import numpy as np

H = 1024
V = 50257
L = 512
NCORES = 8
LC = L // NCORES          # 64 attention rows per core
HC = H // NCORES          # 128 hidden rows per core
VC = 6400                 # padded vocab rows per core (8*6400 = 51200 >= V)
VT = VC // 128            # 50 out_W tiles of [128, 1024] per core
W_BUFS = 21               # out_W streaming pool depth (13 MB SBUF)
NEG = -1.0e30             # bias padding -> exp() == 0

_CACHE = {}


def _build():
    from concourse import bass, bacc, mybir, tile
    from concourse.bass import _add_dep_helper

    fp32 = mybir.dt.float32
    nc = bacc.Bacc("TRN2", target_bir_lowering=False, debug=False,
                   num_devices=NCORES)

    # ---- kernel I/O (per-core shards supplied via in_maps) ----
    emb_row = nc.dram_tensor("emb_row", [1, H], fp32, kind="ExternalInput")
    h_full = nc.dram_tensor("h_full", [1, H], fp32, kind="ExternalInput")
    h_sl = nc.dram_tensor("h_sl", [HC, 1], fp32, kind="ExternalInput")
    enc_c = nc.dram_tensor("enc_c", [LC, H], fp32, kind="ExternalInput")
    attn_w_c = nc.dram_tensor("attn_w_c", [LC, 2 * H], fp32, kind="ExternalInput")
    attn_b_c = nc.dram_tensor("attn_b_c", [LC, 1], fp32, kind="ExternalInput")
    comb_we = nc.dram_tensor("comb_we", [H, H], fp32, kind="ExternalInput")
    comb_wa = nc.dram_tensor("comb_wa", [H, H], fp32, kind="ExternalInput")
    cbt = nc.dram_tensor("cbt", [128, 8], fp32, kind="ExternalInput")
    w_ih_c = nc.dram_tensor("w_ih_c", [3, HC, H], fp32, kind="ExternalInput")
    b_ih_c = nc.dram_tensor("b_ih_c", [3, HC], fp32, kind="ExternalInput")
    w_hh_c = nc.dram_tensor("w_hh_c", [3, HC, H], fp32, kind="ExternalInput")
    b_hh_c = nc.dram_tensor("b_hh_c", [3, HC], fp32, kind="ExternalInput")
    out_w_c = nc.dram_tensor("out_w_c", [VC, H], fp32, kind="ExternalInput")
    out_b_c = nc.dram_tensor("out_b_c", [128, VT], fp32, kind="ExternalInput")
    eye_in = nc.dram_tensor("eye_in", [128, 128], fp32, kind="ExternalInput")

    logits_out = nc.dram_tensor("logits_out", [128, VT], fp32, kind="ExternalOutput")
    hnew_out = nc.dram_tensor("hnew_out", [H], fp32, kind="ExternalOutput")
    attnw_out = nc.dram_tensor("attnw_out", [L], fp32, kind="ExternalOutput")

    AG1W = 1096               # [attn_partial(1024) | sumexp(1) | pad(7) | exp(64)]
    mult = mybir.AluOpType.mult
    add = mybir.AluOpType.add
    AF = mybir.ActivationFunctionType

    with tile.TileContext(nc) as tc:
        with (
            tc.tile_pool(name="cst", bufs=1) as cst,
            tc.tile_pool(name="wpool", bufs=W_BUFS) as wpool,
            tc.tile_pool(name="cwe_p", bufs=2) as cwe_p,
            tc.tile_pool(name="cwa_p", bufs=3) as cwa_p,
            tc.tile_pool(name="psum", bufs=1, space="PSUM") as psum,
            tc.tile_pool(name="dram", bufs=1, space="DRAM") as dram,
        ):
            # ---- collective bounce buffers (DRAM) ----
            ag1_in = dram.tile([AG1W], fp32, tag="ag1i")
            ag1_out = dram.tile([NCORES * AG1W], fp32, tag="ag1o",
                                addr_space="Shared")
            ag3_in = dram.tile([HC], fp32, tag="ag3i")
            ag3_out = dram.tile([H], fp32, tag="ag3o", addr_space="Shared")
            ag4_in = dram.tile([8], fp32, tag="ag4i")
            ag4_out = dram.tile([64], fp32, tag="ag4o", addr_space="Shared")

            # ---- persistent SBUF tiles ----
            cat3 = cst.tile([128, 3 * H], fp32, tag="cat3")   # [emb | h | attn] bcast
            scratch = cst.tile([128, 2 * H], fp32, tag="scratch")
            enc_sb = cst.tile([LC, H + 1], fp32, tag="enc")
            aw_sb = cst.tile([LC, 2 * H], fp32, tag="aw")
            ab_sb = cst.tile([LC, 1], fp32, tag="ab")
            cbt_sb = cst.tile([128, 8], fp32, tag="cbt")
            wih_sb = cst.tile([HC, 3, H], fp32, tag="wih")
            whh_sb = cst.tile([HC, 3, H], fp32, tag="whh")
            bih_sb = cst.tile([HC, 3], fp32, tag="bih")
            bhh_sb = cst.tile([HC, 3], fp32, tag="bhh")
            hsl_sb = cst.tile([HC, 1], fp32, tag="hsl")
            ob_sb = cst.tile([128, VT], fp32, tag="ob")
            row_sb = cst.tile([1, H], fp32, tag="row")        # emb/h staging row
            row2_sb = cst.tile([1, H], fp32, tag="row2")
            xrow = cst.tile([1, H], fp32, tag="xrow")
            hrow = cst.tile([1, H], fp32, tag="hrow")
            xb = cst.tile([128, H], fp32, tag="xb")           # x broadcast
            hb = cst.tile([128, H], fp32, tag="hb")           # h_new broadcast
            ones8 = cst.tile([8, 1], fp32, tag="ones8")
            ones128 = cst.tile([128, 1], fp32, tag="ones128")
            sc_acc = cst.tile([LC, 1], fp32, tag="scacc")
            exps = cst.tile([LC, 1], fp32, tag="exps")
            ar1sb = cst.tile([1, 1096], fp32, tag="ar1sb")
            se_sb = cst.tile([1, 8], fp32, tag="sesb")
            agsb = cst.tile([NCORES, AG1W], fp32, tag="agsb")
            rs_sb = cst.tile([1, 1], fp32, tag="rs")
            rs8_sb = cst.tile([8, 1], fp32, tag="rs8")
            attnn_sb = cst.tile([1, H], fp32, tag="attnn")
            awout_sb = cst.tile([NCORES, LC], fp32, tag="awout")
            comb1 = cst.tile([128, 8], fp32, tag="comb1")
            comb2 = cst.tile([128, 8], fp32, tag="comb2")
            x_sb = cst.tile([128, 8], fp32, tag="xsb")
            gh_sb = cst.tile([HC, 3], fp32, tag="gh")
            gi_sb = cst.tile([HC, 3], fp32, tag="gi")
            gh_acc = cst.tile([HC, 3], fp32, tag="ghacc")
            gi_acc = cst.tile([HC, 3], fp32, tag="giacc")
            lg_acc = cst.tile([128, VT], fp32, tag="lgacc")
            r_g = cst.tile([HC, 1], fp32, tag="rg")
            z_g = cst.tile([HC, 1], fp32, tag="zg")
            t_g = cst.tile([HC, 1], fp32, tag="tg")
            n_g = cst.tile([HC, 1], fp32, tag="ng")
            u_g = cst.tile([HC, 1], fp32, tag="ug")
            hn_sl = cst.tile([HC, 1], fp32, tag="hnsl")
            lg_sb = cst.tile([128, VT], fp32, tag="lg")
            elg_sb = cst.tile([128, VT], fp32, tag="elg")
            sexp_p = cst.tile([128, 1], fp32, tag="sexpp")
            s8_sb = cst.tile([1, 8], fp32, tag="s8")
            stot_sb = cst.tile([1, 1], fp32, tag="stot")
            lse_sb = cst.tile([1, 1], fp32, tag="lse")
            lse128 = cst.tile([128, 1], fp32, tag="lse128")
            outsb = cst.tile([128, VT], fp32, tag="outsb")
            warm_sb = cst.tile([1, 2], fp32, tag="warm")
            eye_sb = cst.tile([128, 128], fp32, tag="eye")
            hnst_sb = cst.tile([1, HC], fp32, tag="hnst")

            psum1 = psum.tile([1, 1536], fp32, tag="p1")
            psum2 = psum.tile([1, 1536], fp32, tag="p2")
            psum3 = psum.tile([1, 1], fp32, tag="p3")
            psum_t = psum.tile([1, 128], fp32, tag="pt")

            # ---- phase 0: load weights / stage broadcasts ----
            nc.scalar.dma_start(out=row_sb[:, :], in_=emb_row[:, :])
            nc.scalar.dma_start(out=row2_sb[:, :], in_=h_full[:, :])
            nc.gpsimd.partition_broadcast(cat3[:, 0:H], row_sb[0:1, :])
            nc.gpsimd.partition_broadcast(cat3[:, H:2 * H], row2_sb[0:1, :])
            nc.sync.dma_start(out=enc_sb[:, 0:H], in_=enc_c[:, :])
            nc.vector.memset(enc_sb[:, H:H + 1], 1.0)
            nc.sync.dma_start(out=aw_sb[:, :], in_=attn_w_c[:, :])
            nc.sync.dma_start(out=ab_sb[:, :], in_=attn_b_c[:, :])
            nc.sync.dma_start(out=cbt_sb[:, :], in_=cbt[:, :])
            wih_ld = nc.sync.dma_start(out=wih_sb[:, :, :],
                              in_=w_ih_c.ap().rearrange("g p c -> p g c"))
            whh_ld = nc.sync.dma_start(out=whh_sb[:, :, :],
                              in_=w_hh_c.ap().rearrange("g p c -> p g c"))
            nc.sync.dma_start(out=bih_sb[:, :],
                              in_=b_ih_c.ap().rearrange("g p -> p g"))
            nc.sync.dma_start(out=bhh_sb[:, :],
                              in_=b_hh_c.ap().rearrange("g p -> p g"))
            nc.scalar.dma_start(out=hsl_sb[:, :], in_=h_sl[:, :])
            ob_ld = nc.sync.dma_start(out=ob_sb[:, :], in_=out_b_c[:, :])
            nc.sync.dma_start(out=eye_sb[:, :], in_=eye_in[:, :])
            nc.vector.memset(ones8[:, :], 1.0)
            nc.vector.memset(ones128[:, :], 1.0)
            # prewarm activation tables off the critical path
            nc.vector.memset(warm_sb[:, 0:1], 1.0)
            for wf in (AF.Exp, AF.Relu, AF.Sigmoid, AF.Tanh, AF.Ln):
                nc.scalar.activation(warm_sb[:, 1:2], warm_sb[:, 0:1], wf)

            # ---- phase 1: attention scores -> exp -> partial attn ----
            nc.vector.scalar_tensor_tensor(
                out=scratch[0:LC, 0:2 * H], in0=aw_sb[:, :], scalar=1.0,
                in1=cat3[0:LC, 0:2 * H], op0=mult, op1=mult,
                accum_out=sc_acc[:, :])
            nc.scalar.activation(exps[:, :], sc_acc[:, :], AF.Exp,
                                 bias=ab_sb[:, :])
            # comb embedded-half (replicated) + gh: fill the dead window
            # before the first collective wakes up
            for b in range(8):
                cwe_t = cwe_p.tile([128, H], fp32, tag="cwe")
                ld = nc.sync.dma_start(out=cwe_t[:, :],
                                       in_=comb_we[b * 128:(b + 1) * 128, :])
                nc.vector.scalar_tensor_tensor(
                    out=scratch[:, 0:H], in0=cwe_t[:, :], scalar=1.0,
                    in1=cat3[:, 0:H], op0=mult, op1=mult,
                    accum_out=comb1[:, b:b + 1])
            for g in range(3):
                nc.vector.scalar_tensor_tensor(
                    out=scratch[:, 0:H], in0=whh_sb[:, g, :], scalar=1.0,
                    in1=cat3[:, H:2 * H], op0=mult, op1=mult,
                    accum_out=gh_acc[:, g:g + 1])
            nc.vector.scalar_tensor_tensor(
                out=gh_sb[:, :], in0=gh_acc[:, :], scalar=1.0,
                in1=bhh_sb[:, :], op0=mult, op1=add)
            nc.vector.scalar_tensor_tensor(
                out=comb1[:, :], in0=comb1[:, :], scalar=1.0,
                in1=cbt_sb[:, :], op0=mult, op1=add)

            # psum1[0, 0:1025] = exp_scores.T @ [enc | 1]
            for n0, n1 in ((0, 512), (512, 1024), (1024, 1025)):
                nc.tensor.matmul(psum1[0:1, n0:n1], exps[:, 0:1],
                                 enc_sb[:, n0:n1], start=True, stop=True)

            nc.tensor.transpose(psum_t[0:1, 0:LC], exps[:, :],
                                 eye_sb[0:LC, 0:LC])
            nc.scalar.copy(ar1sb[:, 0:1025], psum1[0:1, 0:1025])
            nc.vector.memset(ar1sb[:, 1025:1032], 0.0)
            nc.scalar.copy(ar1sb[:, 1032:1096], psum_t[0:1, 0:LC])
            gate = nc.scalar.dma_start(
                out=ag1_in[:].rearrange("(one f) -> one f", one=1),
                in_=ar1sb[:, :])
            nc.gpsimd.collective_compute(
                "AllGather", mybir.AluOpType.bypass,
                replica_groups=[list(range(NCORES))],
                ins=[ag1_in[:].opt()], outs=[ag1_out[:].opt()])
            nc.scalar.dma_start(out=agsb[:, :],
                                in_=ag1_out[:].rearrange("(r f) -> r f", r=NCORES))

            # rank-sum of partials: psum2[0, :1025] = ones.T @ agsb[:, :1025]
            for n0, n1 in ((0, 512), (512, 1024), (1024, 1025)):
                nc.tensor.matmul(psum2[0:1, n0:n1], ones8[:, :],
                                 agsb[:, n0:n1], start=True, stop=True)

            nc.vector.reciprocal(rs_sb[:, :], psum2[0:1, 1024:1025])
            # normalized attn_applied -> broadcast into cat3[:, 2H:3H]
            nc.vector.tensor_scalar_mul(attnn_sb[:, :],
                                        psum2[0:1, 0:H], rs_sb[:, :])
            nc.gpsimd.partition_broadcast(cat3[:, 2 * H:3 * H], attnn_sb[0:1, :])

            # ---- phase 2: combine -> relu -> x slice -> AllGather ----
            # attn-half of comb (replicated weights): full x computed locally
            for b in range(8):
                cwa_t = cwa_p.tile([128, H], fp32, tag="cwa")
                nc.scalar.dma_start(out=cwa_t[:, :],
                                    in_=comb_wa[b * 128:(b + 1) * 128, :])
                nc.vector.scalar_tensor_tensor(
                    out=scratch[:, 0:H], in0=cwa_t[:, :], scalar=1.0,
                    in1=cat3[:, 2 * H:3 * H], op0=mult, op1=mult,
                    accum_out=comb2[:, b:b + 1])
            nc.vector.scalar_tensor_tensor(
                out=comb2[:, :], in0=comb2[:, :], scalar=1.0,
                in1=comb1[:, :], op0=mult, op1=add)
            nc.scalar.activation(x_sb[:, :], comb2[:, :], AF.Relu)
            # x [128, 8] partition-major -> xrow [1, 1024] via PE transposes
            for b in range(8):
                pslot = psum_t[0:1, 0:128] if b % 2 == 0 else psum2[0:1, 0:128]
                nc.tensor.transpose(pslot, x_sb[:, b:b + 1], eye_sb[:, :])
                dst = xrow[0:1, b * 128:(b + 1) * 128]
                if b % 2 == 0:
                    nc.scalar.copy(dst, pslot)
                else:
                    nc.vector.tensor_copy(dst, pslot)
            nc.gpsimd.partition_broadcast(xb[:, :], xrow[0:1, :])

            # ---- phase 3: gi, gates, h_new slice -> AllGather ----
            for g in range(3):
                nc.vector.scalar_tensor_tensor(
                    out=scratch[:, 0:H], in0=wih_sb[:, g, :], scalar=1.0,
                    in1=xb[:, :], op0=mult, op1=mult,
                    accum_out=gi_acc[:, g:g + 1])
            nc.vector.scalar_tensor_tensor(
                out=gi_sb[:, :], in0=gi_acc[:, :], scalar=1.0,
                in1=bih_sb[:, :], op0=mult, op1=add)
            nc.scalar.activation(r_g[:, :], gi_sb[:, 0:1], AF.Sigmoid,
                                 bias=gh_sb[:, 0:1])
            nc.scalar.activation(z_g[:, :], gi_sb[:, 1:2], AF.Sigmoid,
                                 bias=gh_sb[:, 1:2])
            nc.vector.tensor_scalar_mul(t_g[:, :], r_g[:, :], gh_sb[:, 2:3])
            nc.scalar.activation(n_g[:, :], gi_sb[:, 2:3], AF.Tanh,
                                 bias=t_g[:, :])
            nc.vector.tensor_scalar_sub(u_g[:, :], hsl_sb[:, :], n_g[:, :])
            # h_new = z*(h-n) + n
            nc.vector.scalar_tensor_tensor(
                out=hn_sl[:, :], in0=z_g[:, :], scalar=u_g[:, :],
                in1=n_g[:, :], op0=mult, op1=add)
            nc.tensor.transpose(psum_t[0:1, 0:HC], hn_sl[:, :], eye_sb[:, :])
            nc.scalar.copy(hnst_sb[:, :], psum_t[0:1, 0:HC])
            nc.scalar.dma_start(
                out=ag3_in[:].rearrange("(one f) -> one f", one=1),
                in_=hnst_sb[:, :])
            nc.gpsimd.collective_compute(
                "AllGather", mybir.AluOpType.bypass,
                replica_groups=[list(range(NCORES))],
                ins=[ag3_in[:].opt()], outs=[ag3_out[:].opt()])
            nc.scalar.dma_start(out=hrow[0:1, :],
                                in_=ag3_out[:].rearrange("(one f) -> one f", one=1))
            nc.gpsimd.partition_broadcast(hb[:, :], hrow[0:1, :])

            # ---- phase 4: vocab-shard logits via fused mul+reduce ----
            for t in range(VT):
                wt = wpool.tile([128, H], fp32, tag="wt")
                wld = nc.sync.dma_start(out=wt[:, :],
                                        in_=out_w_c[t * 128:(t + 1) * 128, :])
                _add_dep_helper(wld.ins, gate.ins, True,
                                "stream after AG1 input to keep queues clear")
                nc.vector.scalar_tensor_tensor(
                    out=scratch[:, 0:H], in0=wt[:, :], scalar=1.0,
                    in1=hb[:, :], op0=mult, op1=mult,
                    accum_out=lg_acc[:, t:t + 1])
            nc.vector.scalar_tensor_tensor(
                out=lg_sb[:, :], in0=lg_acc[:, :], scalar=1.0,
                in1=ob_sb[:, :], op0=mult, op1=add)

            for ld in (wih_ld, whh_ld, ob_ld):
                _add_dep_helper(ld.ins, gate.ins, True,
                                "bulk load after AG1 input to keep queues clear")

            # attn_weights / h_new outputs (created late => low priority,
            # they fill idle engine time during the logits phase)
            nc.gpsimd.partition_broadcast(rs8_sb[:, :], rs_sb[0:1, :])
            nc.vector.tensor_scalar_mul(awout_sb[:, :],
                                        agsb[:, 1032:1096], rs8_sb[:, :])
            nc.scalar.dma_start(out=attnw_out.ap().rearrange("(p f) -> p f", p=NCORES),
                                in_=awout_sb[:, :])
            nc.scalar.dma_start(out=hnew_out[:], in_=ag3_out[:])

            # local sum(exp(logits)) -> AllGather -> global logsumexp
            nc.scalar.activation(elg_sb[:, :], lg_sb[:, :], AF.Exp,
                                 accum_out=sexp_p[:, :])
            nc.tensor.matmul(psum3[0:1, 0:1], ones128[:, :], sexp_p[:, :],
                             start=True, stop=True)
            nc.scalar.copy(se_sb[:, 0:1], psum3[0:1, 0:1])
            nc.vector.memset(se_sb[:, 1:8], 0.0)
            nc.scalar.dma_start(
                out=ag4_in[:].rearrange("(one f) -> one f", one=1),
                in_=se_sb[:, :])
            nc.gpsimd.collective_compute(
                "AllGather", mybir.AluOpType.bypass,
                replica_groups=[list(range(NCORES))],
                ins=[ag4_in[:].opt()], outs=[ag4_out[:].opt()])
            # rank r's local sum-exp sits at element r*8 of ag4_out
            nc.scalar.dma_start(
                out=s8_sb[0:1, :],
                in_=ag4_out[:].rearrange("(r f) -> f r", f=8)[0:1, :])
            nc.vector.reduce_sum(stot_sb[:, :], s8_sb[:, :],
                                 axis=mybir.AxisListType.X)
            nc.scalar.activation(lse_sb[:, :], stot_sb[:, :], AF.Ln)
            nc.gpsimd.partition_broadcast(lse128[:, :], lse_sb[0:1, :])
            nc.vector.tensor_scalar_sub(outsb[:, :], lg_sb[:, :], lse128[:, :])
            nc.scalar.dma_start(out=logits_out[:, :], in_=outsb[:, :])

    nc.compile()
    return nc


def _get_nc():
    if "nc" not in _CACHE:
        _CACHE["nc"] = _build()
    return _CACHE["nc"]


def _shard_inputs(input_idx, hidden, encoder_output, emb, attn_W, attn_b,
                  comb_W, comb_b, W_ih, W_hh, b_ih, b_hh, out_W, out_b):
    f32 = np.float32
    idx = int(np.asarray(input_idx).reshape(-1)[0])
    emb_row = np.ascontiguousarray(np.asarray(emb)[idx]).reshape(1, H).astype(f32)
    h = np.asarray(hidden, dtype=f32).reshape(1, H)
    enc = np.asarray(encoder_output, dtype=f32)
    attn_W = np.asarray(attn_W, dtype=f32)
    attn_b = np.asarray(attn_b, dtype=f32)
    comb_W = np.asarray(comb_W, dtype=f32)
    comb_b = np.asarray(comb_b, dtype=f32)
    W_ih = np.asarray(W_ih, dtype=f32).reshape(3, H, H)
    W_hh = np.asarray(W_hh, dtype=f32).reshape(3, H, H)
    b_ih = np.asarray(b_ih, dtype=f32).reshape(3, H)
    b_hh = np.asarray(b_hh, dtype=f32).reshape(3, H)
    out_W = np.asarray(out_W, dtype=f32)
    out_b = np.asarray(out_b, dtype=f32)

    comb_we = np.ascontiguousarray(comb_W[:, :H])
    comb_wa = np.ascontiguousarray(comb_W[:, H:])
    cbt_host = np.ascontiguousarray(comb_b.reshape(8, 128).T)

    out_W_pad = np.zeros((NCORES * VC, H), dtype=f32)
    out_W_pad[:V] = out_W
    out_b_pad = np.full((NCORES * VC,), NEG, dtype=f32)
    out_b_pad[:V] = out_b

    in_maps = []
    for c in range(NCORES):
        ls, le = c * LC, (c + 1) * LC
        hs, he = c * HC, (c + 1) * HC
        vs, ve = c * VC, (c + 1) * VC
        in_maps.append({
            "emb_row": emb_row,
            "h_full": h,
            "h_sl": np.ascontiguousarray(h[0, hs:he]).reshape(HC, 1),
            "enc_c": np.ascontiguousarray(enc[ls:le]),
            "attn_w_c": np.ascontiguousarray(attn_W[ls:le]),
            "attn_b_c": np.ascontiguousarray(attn_b[ls:le]).reshape(LC, 1),
            "comb_we": comb_we,
            "comb_wa": comb_wa,
            "cbt": cbt_host,
            "w_ih_c": np.ascontiguousarray(W_ih[:, hs:he, :]),
            "b_ih_c": np.ascontiguousarray(b_ih[:, hs:he]),
            "w_hh_c": np.ascontiguousarray(W_hh[:, hs:he, :]),
            "b_hh_c": np.ascontiguousarray(b_hh[:, hs:he]),
            "out_w_c": np.ascontiguousarray(out_W_pad[vs:ve]),
            "out_b_c": np.ascontiguousarray(out_b_pad[vs:ve].reshape(VT, 128).T),
            "eye_in": np.eye(128, dtype=f32),
        })
    return in_maps


def kernel(**inputs):
    from concourse import bass_utils

    nc = _get_nc()
    in_maps = _shard_inputs(**inputs)
    trace = bool(int(os.environ.get("KERNEL_PROFILE", "0")))
    res = bass_utils.run_bass_kernel_spmd(
        nc, in_maps, core_ids=list(range(NCORES)), trace=trace)
    if trace:
        _CACHE["last_result"] = res

    logits = np.concatenate(
        [res.results[c]["logits_out"].reshape(128, VT).T.reshape(-1)
         for c in range(NCORES)])
    out = logits[:V].reshape(1, V).astype(np.float32)
    h_new = res.results[0]["hnew_out"].reshape(1, 1, H).astype(np.float32)
    attn_w = res.results[0]["attnw_out"].reshape(1, L).astype(np.float32)
    return out, h_new, attn_w
